# revision 36
# baseline (speedup 1.0000x reference)
"""Gated attention-with-pair-bias kernel for one TRN2 chip (8 NeuronCores).

Reference computation (per batch b):
  q = q_x @ Wq.T ; k = k_x @ Wk.T ; v = v_x @ Wv.T          (heads H=8, DH=32)
  logits = q k^T / sqrt(DH) + bias_mask + bias_pair          [B,H,S,S]
  probs  = softmax(logits)                                   (S = 2048)
  o      = (probs @ v) * sigmoid(q_x @ Wg.T + bg)
  out    = o @ Wo.T + bo

Sharding: sequence-parallel over the Q dimension. Core i computes output rows
[i*256, (i+1)*256) for both batches and all heads; K/V are replicated. Outputs
are disjoint so no collectives are needed.

Device layout: logits are computed TRANSPOSED ([ks, qs], ks on partitions) so
that softmax'd probs feed the PV matmul directly with no transposes.
 - QK^T: 4-way row-packed matmuls (contraction DH=32 -> 4 heads concurrent,
   each head's output in its own PSUM bank)
 - pair bias: shipped as exp(bias_pair) in bf16 and multiplied into the
   exp'd logits on the vector engine (softmax(a+b) ~ exp(a)*exp(b) / sum)
 - bias_mask: per-partition bias of the ACT exp instruction
 - softmax: max-subtraction skipped (logits are O(10), exp is safe in f32),
   denominator comes from an extra all-ones column appended to V (M=33 PV
   matmuls)
"""

import os
import numpy as np
import ml_dtypes

BF16 = ml_dtypes.bfloat16

B, S, C = 2, 2048, 256
H, DH = 8, 32
N_CORES = 8
QS = S // N_CORES          # 256 q rows per core
KST = S // 128             # 16 k-tiles of 128

_CACHE = {}
LAST_RESULT = None
# Column-grouped PV accumulators (2 PSUM banks instead of 4) were tried and
# work for single matmuls on HW, but corrupt the 16-step accumulation
# (CoreSim does not model column tile_position at all); keep the 4-bank
# layout.
PV_COL_PACK = False


def _build_graph():
    import concourse.bass as bass
    import concourse.mybir as mybir
    import concourse.tile as tile
    from concourse import bacc
    from concourse.masks import make_identity
    from contextlib import ExitStack

    F32 = mybir.dt.float32
    BF = mybir.dt.bfloat16
    Sig = mybir.ActivationFunctionType.Sigmoid
    Exp = mybir.ActivationFunctionType.Exp

    nc = bacc.Bacc()

    qxT_d = nc.declare_dram_parameter("qxT", [B, C, QS], BF, isOutput=False)
    kxT_d = nc.declare_dram_parameter("kxT", [B, C, S], BF, isOutput=False)
    vxT_d = nc.declare_dram_parameter("vxT", [B, C, S], BF, isOutput=False)
    wqT_d = nc.declare_dram_parameter("wqT", [C, C], BF, isOutput=False)
    wkT_d = nc.declare_dram_parameter("wkT", [C, C], BF, isOutput=False)
    wvT_d = nc.declare_dram_parameter("wvT", [C, C], BF, isOutput=False)
    wgT_d = nc.declare_dram_parameter("wgT", [C, C], BF, isOutput=False)
    woT_d = nc.declare_dram_parameter("woT", [C, C], BF, isOutput=False)
    bgt_d = nc.declare_dram_parameter("bgt", [128, 2], F32, isOutput=False)
    bo_d = nc.declare_dram_parameter("bo", [1, C], F32, isOutput=False)
    maskT_d = nc.declare_dram_parameter("maskT", [B, 128, KST], F32, isOutput=False)
    biasT_d = nc.declare_dram_parameter("biasT", [B, KST, 128, 4, 2, QS], BF, isOutput=False)
    out_d = nc.declare_dram_parameter("out", [B, QS, C], F32, isOutput=True)

    mm = nc.tensor.matmul

    with ExitStack() as ctx:
        tc = ctx.enter_context(tile.TileContext(nc))
        const = ctx.enter_context(tc.tile_pool(name="const", bufs=1))
        acts = ctx.enter_context(tc.tile_pool(name="acts", bufs=1))
        biasP = ctx.enter_context(tc.tile_pool(name="biasP", bufs=5))
        probsP = ctx.enter_context(tc.tile_pool(name="probsP", bufs=5))
        epiP = ctx.enter_context(tc.tile_pool(name="epiP", bufs=2))
        dramP = ctx.enter_context(tc.tile_pool(name="dramP", bufs=2, space="DRAM"))
        # PSUM budget is 8 banks: one 4-bank QK tile (psB, bufs=1), the PV
        # accumulators (2 banks when column-packed), and a 2-bank aux pool
        # for projection / output-projection psums so they never contend
        # with the PV accumulators.
        psB = ctx.enter_context(tc.tile_pool(name="psB", bufs=1, space="PSUM"))
        psPV = ctx.enter_context(tc.tile_pool(name="psPV", bufs=(2 if PV_COL_PACK else 4), space="PSUM"))
        if PV_COL_PACK:
            psAux = ctx.enter_context(tc.tile_pool(name="psAux", bufs=2, space="PSUM"))
            aux_tile = lambda: psAux.tile([128, 512], F32, name="proj", tag="aux")
        else:
            aux_tile = lambda: psPV.tile([128, 512], F32, name="proj", tag="pv")

        # ---- constants ----
        w_sb = {}
        for name, d in [("wq", wqT_d), ("wk", wkT_d), ("wg", wgT_d), ("wv", wvT_d)]:
            t = const.tile([128, 2, C], BF, name=name, tag=name)
            nc.gpsimd.dma_start(out=t[:], in_=d.rearrange("(cc p) o -> p cc o", p=128))
            w_sb[name] = t
        # wo shipped head-major: [H, DH, C] -> SBUF [DH, H, C] so the output
        # projection can contract straight out of the [33, 2048] osb layout
        woh_sb = const.tile([DH, H, C], BF, name="woh", tag="woh")
        nc.gpsimd.dma_start(out=woh_sb[:], in_=woT_d.rearrange("(h d) c -> d h c", d=DH))
        bgt_sb = const.tile([128, 2], F32, name="bgt", tag="bgt")
        nc.gpsimd.dma_start(out=bgt_sb[:], in_=bgt_d[:])
        bo_sb = const.tile([128, C], F32, name="bo", tag="bo")
        nc.gpsimd.dma_start(out=bo_sb[:], in_=bo_d[0:1, :].broadcast_to([128, C]))
        maskT_sb = const.tile([128, B, KST], F32, name="maskT", tag="maskT")
        nc.gpsimd.dma_start(out=maskT_sb[:], in_=maskT_d.rearrange("b p k -> p b k"))

        # ---- activations in ----
        qxT_sb, kxT_sb, vxT_sb = {}, {}, {}
        kT_sb, qT_sb, gT_sb, gT2_sb, v_sb = {}, {}, {}, {}, {}
        for b in range(B):
            qxT_sb[b] = acts.tile([128, 2, QS], BF, name=f"qx{b}", tag=f"qx{b}")
            nc.sync.dma_start(out=qxT_sb[b][:], in_=qxT_d[b].rearrange("(cc p) q -> p cc q", p=128))
            # split the big activation loads per c-chunk so dependent
            # projection tiles can start before the whole tensor lands
            kxT_sb[b] = acts.tile([128, 2, S], BF, name=f"kx{b}", tag=f"kx{b}")
            vxT_sb[b] = acts.tile([128, 2, S], BF, name=f"vx{b}", tag=f"vx{b}")
            for cc in range(2):
                nc.sync.dma_start(out=kxT_sb[b][:, cc, :], in_=kxT_d[b, cc * 128:(cc + 1) * 128, :])
                nc.sync.dma_start(out=vxT_sb[b][:, cc, :], in_=vxT_d[b, cc * 128:(cc + 1) * 128, :])
            kT_sb[b] = [acts.tile([128, S], BF, name=f"kT{b}_{oc}", tag=f"kT{b}_{oc}") for oc in range(2)]
            qT_sb[b] = acts.tile([128, 2, QS], BF, name=f"qT{b}", tag=f"qT{b}")
            gT_sb[b] = acts.tile([128, 2, QS], BF, name=f"gT{b}", tag=f"gT{b}")
            v_sb[b] = [acts.tile([128, H, DH + 1], BF, name=f"v{b}_{kst}", tag=f"v{b}_{kst}") for kst in range(KST)]

        def emit_gate(b):
            # gate projection: only needed by the epilogue, so it is emitted
            # after the batch's attention loop to keep the startup prefix
            # short. gT2 is the head-major copy matching the osb layout.
            for oc in range(2):
                ps = aux_tile()
                for cc in range(2):
                    mm(ps[:, :QS], lhsT=w_sb["wg"][:, cc, oc * 128:(oc + 1) * 128],
                       rhs=qxT_sb[b][:, cc, :], start=(cc == 0), stop=(cc == 1))
                nc.scalar.activation(gT_sb[b][:, oc, :], ps[:, :QS], Sig,
                                     bias=bgt_sb[:, oc:oc + 1])
            gT2_sb[b] = acts.tile([DH, H, QS], BF, name=f"gT2{b}", tag=f"gT2{b}")
            for h in range(H):
                nc.gpsimd.dma_start(out=gT2_sb[b][:, h, :],
                                    in_=gT_sb[b][32 * (h % 4):32 * (h % 4) + 32, h // 4, :])

        # ---- phase 0: projections ----
        for b in range(B):
            for oc in range(2):  # q
                ps = aux_tile()
                for cc in range(2):
                    mm(ps[:, :QS], lhsT=w_sb["wq"][:, cc, oc * 128:(oc + 1) * 128],
                       rhs=qxT_sb[b][:, cc, :], start=(cc == 0), stop=(cc == 1))
                nc.vector.tensor_copy(qT_sb[b][:, oc, :], ps[:, :QS])
            for oc in range(2):  # k
                for ns in range(4):
                    ps = aux_tile()
                    for cc in range(2):
                        mm(ps[:, :512], lhsT=w_sb["wk"][:, cc, oc * 128:(oc + 1) * 128],
                           rhs=kxT_sb[b][:, cc, ns * 512:(ns + 1) * 512],
                           start=(cc == 0), stop=(cc == 1))
                    # ACT is idle during the projection phase; route the k
                    # copies there so DVE keeps only the v/q copies
                    nc.scalar.copy(kT_sb[b][oc][:, ns * 512:(ns + 1) * 512], ps[:, :512])
            for kst2 in range(KST // 2):  # v (+ ones column per head)
                ps = aux_tile()
                for half in range(2):
                    kst = 2 * kst2 + half
                    for cc in range(2):
                        mm(ps[:, half * C:(half + 1) * C],
                           lhsT=vxT_sb[b][:, cc, kst * 128:(kst + 1) * 128],
                           rhs=w_sb["wv"][:, cc, :],
                           start=(half == 0 and cc == 0), stop=(half == 1 and cc == 1),
                           skip_group_check=True)
                for half in range(2):
                    kst = 2 * kst2 + half
                    va = v_sb[b][kst]
                    nc.vector.tensor_copy(va[:, :, 0:DH],
                                          ps[:, half * C:(half + 1) * C].rearrange("p (h c) -> p h c", c=DH))
                    nc.vector.memset(va[:, :, DH:DH + 1], 1.0)

        # ---- attention ----
        ofin = {}
        for b in range(B):
            # PV accumulators. Column-packed: 2 one-bank tiles, head
            # h = g*4+s at rows (s%2)*64..+33 and free (s//2)*256 of tile g
            # (4 heads per bank; concurrent col-group writes hit different
            # partition groups, which the HW supports). Fallback: 4 tiles,
            # head 2p+hs at rows 0:33, free hs*256 of tile p.
            n_pv = 2 if PV_COL_PACK else 4
            pvt = [psPV.tile([128, 512], F32, name=f"pv{p}", tag="pv") for p in range(n_pv)]

            def pv_slot(h):
                if PV_COL_PACK:
                    g, s = h // 4, h % 4
                    return pvt[g], (s % 2) * 64, (s // 2) * 256, (s == 0), (s == 3)
                pair, hs = h // 2, h % 2
                return pvt[pair], 0, hs * 256, (hs == 0), (hs == 1)
            # Probs layout: head h = quad*4 + j lives at free offset
            # j*512 + quad*256 of the [128, 2048] qk/probs tiles, so the 4
            # concurrently-active row-tiled QK matmuls (row groups 32j) each
            # write a DIFFERENT psum bank (concurrent same-bank PE writes
            # hang the chip); the two quads reuse the same row groups and
            # therefore serialize on the PE.
            def emit_pv(probs, kst):
                # PV accumulates across the whole kst loop. At kst==0 the
                # slot-0 matmul's start=True zeroes its whole bank, which
                # races any concurrently-running col-group-64 matmul on the
                # same bank; issue order [s0, s2, s1, s3] keeps the odd col
                # groups strictly after the zeroing matmul has drained.
                heads = range(H)
                if PV_COL_PACK and kst == 0:
                    heads = [g * 4 + s for g in range(2) for s in (0, 2, 1, 3)]
                for h in heads:
                    tile_, row0, foff, first, last = pv_slot(h)
                    off = (h % 4) * 512 + (h // 4) * QS
                    mm(tile_[row0:row0 + 33, foff:foff + QS],
                       lhsT=v_sb[b][kst][:, h, :],
                       rhs=probs[:, off:off + QS],
                       start=(kst == 0 and first),
                       stop=(kst == KST - 1 and last),
                       tile_position=(0, row0),
                       skip_group_check=True)

            prev = None
            for kst in range(KST):
                bt = biasP.tile([128, 2048], BF, name="bias", tag="bias")
                nc.sync.dma_start(out=bt[:], in_=biasT_d[b, kst].rearrange("p a b q -> p (a b q)"))
                qk = psB.tile([128, 2048], F32, name="qk", tag="qk")
                for quad in range(2):
                    for j in range(4):
                        off = j * 512 + quad * QS
                        mm(qk[:, off:off + QS],
                           lhsT=kT_sb[b][quad][32 * j:32 * j + 32, kst * 128:(kst + 1) * 128],
                           rhs=qT_sb[b][32 * j:32 * j + 32, quad, :],
                           start=(quad == 0), stop=(quad == 1), tile_position=(32 * j, 0),
                           skip_group_check=True)
                probs = probsP.tile([128, 2048], BF, name="probs", tag="probs")
                # exp in two half-tile ACT instructions: banks 0-1 (heads
                # j=0,1) then banks 2-3. QK(kst+1)'s j=0,1 matmuls only WAR
                # on the first half, so they run during the second ACT and
                # drop out of the ACT->QK->ACT critical chain.
                for half in range(2):
                    sl = slice(half * 1024, (half + 1) * 1024)
                    nc.scalar.activation(probs[:, sl], qk[:, sl], Exp,
                                         bias=maskT_sb[:, b, kst:kst + 1])
                    nc.vector.tensor_mul(probs[:, sl], probs[:, sl], bt[:, sl])
                # PV for the previous kst is emitted after this kst's QK/exp
                # so the PE can start QK(kst+1) the moment ACT(kst) frees the
                # qk tile instead of sitting behind the probs multiply.
                if prev is not None:
                    emit_pv(*prev)
                prev = (probs, kst)
            emit_pv(*prev)
            emit_gate(b)

            # ---- epilogue part 1 for batch b (no PSUM, no sync queue) ----
            # Frees the pv psum slots immediately so the next batch's
            # attention can start; DMAs go on the gpsimd queue so the sync
            # queue keeps feeding the next batch's bias tiles.
            # osb[0:33, h*256:(h+1)*256] = head h's [o^T ; l] block
            osb = epiP.tile([33, 2048], F32, name=f"osb{b}", tag=f"osb{b}")
            if PV_COL_PACK:
                for h in range(H):
                    tile_, row0, foff, _, _ = pv_slot(h)
                    nc.vector.tensor_copy(osb[0:33, h * QS:(h + 1) * QS],
                                          tile_[row0:row0 + 33, foff:foff + QS])
            else:
                for pair in range(4):
                    nc.vector.tensor_copy(osb[0:33, pair * 512:(pair + 1) * 512], pvt[pair][0:33, :])
            # l lives in row 32 as one contiguous [1, 8*256] stripe: bounce it
            # through DRAM to fold it to [8, 256], reciprocal, bounce back,
            # then broadcast the whole 2048-wide stripe to all 32 d-rows.
            lb = dramP.tile([1, H * QS], F32, name=f"lb{b}", tag="lb")
            nc.sync.dma_start(out=lb[:], in_=osb[32:33, :])
            lsb = epiP.tile([H, QS], F32, name="lsb", tag="lsb")
            nc.sync.dma_start(out=lsb[:], in_=lb[0].rearrange("(h q) -> h q", q=QS))
            nc.vector.reciprocal(lsb[:], lsb[:])
            lb2 = dramP.tile([H, QS], F32, name=f"lb2{b}", tag="lb2")
            nc.sync.dma_start(out=lb2[:], in_=lsb[:])
            rep = epiP.tile([DH, H * QS], F32, name="rep", tag="rep")
            nc.sync.dma_start(out=rep[:],
                              in_=lb2.rearrange("h q -> (h q)")[None, :].broadcast_to([DH, H * QS]))
            nc.vector.tensor_mul(osb[0:DH, :], osb[0:DH, :], rep[:])
            of = epiP.tile([DH, H * QS], BF, name=f"ofin{b}", tag=f"ofin{b}")
            nc.vector.tensor_mul(of[:], osb[0:DH, :], gT2_sb[b].rearrange("d h q -> d (h q)"))
            ofin[b] = of

        # ---- output projections for both batches (pv slots are free) ----
        # out[qs, c] = sum_h ofin[0:DH, h*QS+qs].T @ woh[:, h, :]
        for b in range(B):
            for qc in range(2):
                ps = psB.tile([128, 2048], F32, name="ops", tag="qk")
                for h in range(H):
                    mm(ps[:, :C], lhsT=ofin[b][:, h * QS + qc * 128: h * QS + (qc + 1) * 128],
                       rhs=woh_sb[:, h, :], start=(h == 0), stop=(h == H - 1))
                outsb = epiP.tile([128, C], F32, name="outsb", tag="outsb")
                nc.vector.tensor_add(outsb[:], ps[:, :C], bo_sb[:])
                nc.sync.dma_start(out=out_d[b, qc * 128:(qc + 1) * 128, :], in_=outsb[:])

    nc.finalize()
    return nc


def _prep_inputs(q_x, k_x, v_x, bias_mask, bias_pair, Wq, Wk, Wv, Wg, bg, Wo, bo):
    scale = np.float32(1.0 / np.sqrt(DH))
    wqT = (Wq.astype(np.float32) * scale).T.copy().astype(BF16)
    wkT = Wk.T.copy().astype(BF16)
    wvT = Wv.T.copy().astype(BF16)
    wgT = Wg.T.copy().astype(BF16)
    woT = Wo.T.copy().astype(BF16)
    bgt = bg.astype(np.float32).reshape(2, 128).T.copy()
    bo2 = bo.astype(np.float32).reshape(1, C).copy()
    maskT = bias_mask.astype(np.float32).reshape(B, KST, 128).transpose(0, 2, 1).copy()
    kxT = k_x.transpose(0, 2, 1).copy().astype(BF16)
    vxT = v_x.transpose(0, 2, 1).copy().astype(BF16)

    # per-core tensors
    in_maps = []
    # biasT[core][b, kst, p, j, quad, qs] = exp(bias_pair)[b, h=quad*4+j,
    #                                                      core*QS+qs, kst*128+p]
    bp = bias_pair.transpose(0, 3, 1, 2)  # [b, k, h, q] view
    for i in range(N_CORES):
        qsl = slice(i * QS, (i + 1) * QS)
        qxT = q_x[:, qsl, :].transpose(0, 2, 1).copy().astype(BF16)
        biasT = np.exp(np.ascontiguousarray(bp[:, :, :, qsl]), dtype=np.float32)
        biasT = biasT.reshape(B, KST, 128, 2, 4, QS).swapaxes(4, 3).astype(BF16)
        biasT = np.ascontiguousarray(biasT)
        in_maps.append({
            "qxT": qxT, "kxT": kxT, "vxT": vxT,
            "wqT": wqT, "wkT": wkT, "wvT": wvT, "wgT": wgT, "woT": woT,
            "bgt": bgt, "bo": bo2, "maskT": maskT, "biasT": biasT,
        })
    return in_maps


def kernel(q_x, k_x, v_x, bias_mask, bias_pair, Wq, Wk, Wv, Wg, bg, Wo, bo):
    global LAST_RESULT
    from concourse.bass_utils import run_bass_kernel_spmd

    args = [np.asarray(a) for a in
            (q_x, k_x, v_x, bias_mask, bias_pair, Wq, Wk, Wv, Wg, bg, Wo, bo)]
    if "nc" not in _CACHE:
        _CACHE["nc"] = _build_graph()
    nc = _CACHE["nc"]
    in_maps = _prep_inputs(*args)
    res = run_bass_kernel_spmd(
        nc, in_maps, core_ids=list(range(N_CORES)),
        trace=bool(os.environ.get("KERNEL_TRACE")),
    )
    LAST_RESULT = res
    out = np.concatenate([res.results[i]["out"] for i in range(N_CORES)], axis=1)
    return out.astype(np.float32)


# revision 39
# speedup vs baseline: 1.1174x; 1.1174x over previous
"""Gated attention-with-pair-bias kernel for one TRN2 chip (8 NeuronCores).

Reference computation (per batch b):
  q = q_x @ Wq.T ; k = k_x @ Wk.T ; v = v_x @ Wv.T          (heads H=8, DH=32)
  logits = q k^T / sqrt(DH) + bias_mask + bias_pair          [B,H,S,S]
  probs  = softmax(logits)                                   (S = 2048)
  o      = (probs @ v) * sigmoid(q_x @ Wg.T + bg)
  out    = o @ Wo.T + bo

Sharding: sequence-parallel over the Q dimension. Core i computes output rows
[i*256, (i+1)*256) for both batches and all heads; K/V are replicated. Outputs
are disjoint so no collectives are needed.

Device layout: logits are computed TRANSPOSED ([ks, qs], ks on partitions) so
that softmax'd probs feed the PV matmul directly with no transposes.
 - QK^T: 4-way row-packed matmuls (contraction DH=32 -> 4 heads concurrent,
   each head's output in its own PSUM bank)
 - pair bias: shipped as exp(bias_pair) in bf16 and multiplied into the
   exp'd logits on the vector engine (softmax(a+b) ~ exp(a)*exp(b) / sum)
 - bias_mask: per-partition bias of the ACT exp instruction
 - softmax: max-subtraction skipped (logits are O(10), exp is safe in f32),
   denominator comes from an extra all-ones column appended to V (M=33 PV
   matmuls)
"""

import os
import numpy as np
import ml_dtypes

BF16 = ml_dtypes.bfloat16

B, S, C = 2, 2048, 256
H, DH = 8, 32
N_CORES = 8
QS = S // N_CORES          # 256 q rows per core
KST = S // 128             # 16 k-tiles of 128

_CACHE = {}
LAST_RESULT = None
# Column-grouped PV accumulators (2 PSUM banks instead of 4) were tried and
# work for single matmuls on HW, but corrupt the 16-step accumulation
# (CoreSim does not model column tile_position at all); keep the 4-bank
# layout.
PV_COL_PACK = False


def _build_graph():
    import concourse.bass as bass
    import concourse.mybir as mybir
    import concourse.tile as tile
    from concourse import bacc
    from concourse.masks import make_identity
    from contextlib import ExitStack

    F32 = mybir.dt.float32
    BF = mybir.dt.bfloat16
    Sig = mybir.ActivationFunctionType.Sigmoid
    Exp = mybir.ActivationFunctionType.Exp

    nc = bacc.Bacc()

    qxT_d = nc.declare_dram_parameter("qxT", [B, C, QS], BF, isOutput=False)
    kxT_d = nc.declare_dram_parameter("kxT", [B, C, S], BF, isOutput=False)
    vxT_d = nc.declare_dram_parameter("vxT", [B, C, S], BF, isOutput=False)
    wqT_d = nc.declare_dram_parameter("wqT", [C, C], BF, isOutput=False)
    wkT_d = nc.declare_dram_parameter("wkT", [C, C], BF, isOutput=False)
    wvT_d = nc.declare_dram_parameter("wvT", [C, C], BF, isOutput=False)
    wgT_d = nc.declare_dram_parameter("wgT", [C, C], BF, isOutput=False)
    woT_d = nc.declare_dram_parameter("woT", [C, C], BF, isOutput=False)
    bgt_d = nc.declare_dram_parameter("bgt", [128, 2], F32, isOutput=False)
    bo_d = nc.declare_dram_parameter("bo", [1, C], F32, isOutput=False)
    maskT_d = nc.declare_dram_parameter("maskT", [B, 128, KST], F32, isOutput=False)
    biasT_d = nc.declare_dram_parameter("biasT", [B, KST, 128, 4, 2, QS], BF, isOutput=False)
    out_d = nc.declare_dram_parameter("out", [B, QS, C], F32, isOutput=True)

    mm = nc.tensor.matmul

    with ExitStack() as ctx:
        tc = ctx.enter_context(tile.TileContext(nc))
        const = ctx.enter_context(tc.tile_pool(name="const", bufs=1))
        acts = ctx.enter_context(tc.tile_pool(name="acts", bufs=1))
        biasP = ctx.enter_context(tc.tile_pool(name="biasP", bufs=5))
        probsP = ctx.enter_context(tc.tile_pool(name="probsP", bufs=5))
        epiP = ctx.enter_context(tc.tile_pool(name="epiP", bufs=2))
        dramP = ctx.enter_context(tc.tile_pool(name="dramP", bufs=2, space="DRAM"))
        # PSUM budget is 8 banks: one 4-bank QK tile (psB, bufs=1), the PV
        # accumulators (2 banks when column-packed), and a 2-bank aux pool
        # for projection / output-projection psums so they never contend
        # with the PV accumulators.
        psB = ctx.enter_context(tc.tile_pool(name="psB", bufs=1, space="PSUM"))
        psPV = ctx.enter_context(tc.tile_pool(name="psPV", bufs=(2 if PV_COL_PACK else 4), space="PSUM"))
        if PV_COL_PACK:
            psAux = ctx.enter_context(tc.tile_pool(name="psAux", bufs=2, space="PSUM"))
            aux_tile = lambda: psAux.tile([128, 512], F32, name="proj", tag="aux")
        else:
            aux_tile = lambda: psPV.tile([128, 512], F32, name="proj", tag="pv")

        # ---- constants ----
        w_sb = {}
        for name, d in [("wq", wqT_d), ("wk", wkT_d), ("wg", wgT_d), ("wv", wvT_d)]:
            t = const.tile([128, 2, C], BF, name=name, tag=name)
            nc.gpsimd.dma_start(out=t[:], in_=d.rearrange("(cc p) o -> p cc o", p=128))
            w_sb[name] = t
        # wo shipped head-major: [H, DH, C] -> SBUF [DH, H, C] so the output
        # projection can contract straight out of the [33, 2048] osb layout
        woh_sb = const.tile([DH, H, C], BF, name="woh", tag="woh")
        nc.gpsimd.dma_start(out=woh_sb[:], in_=woT_d.rearrange("(h d) c -> d h c", d=DH))
        bgt_sb = const.tile([128, 2], F32, name="bgt", tag="bgt")
        nc.gpsimd.dma_start(out=bgt_sb[:], in_=bgt_d[:])
        bo_sb = const.tile([128, C], F32, name="bo", tag="bo")
        nc.gpsimd.dma_start(out=bo_sb[:], in_=bo_d[0:1, :].broadcast_to([128, C]))
        maskT_sb = const.tile([128, B, KST], F32, name="maskT", tag="maskT")
        nc.gpsimd.dma_start(out=maskT_sb[:], in_=maskT_d.rearrange("b p k -> p b k"))

        # ---- activations in ----
        qxT_sb, kxT_sb, vxT_sb = {}, {}, {}
        kT_sb, qT_sb, gT_sb, gT2_sb, v_sb = {}, {}, {}, {}, {}
        for b in range(B):
            qxT_sb[b] = acts.tile([128, 2, QS], BF, name=f"qx{b}", tag=f"qx{b}")
            nc.sync.dma_start(out=qxT_sb[b][:], in_=qxT_d[b].rearrange("(cc p) q -> p cc q", p=128))
            # split the big activation loads per c-chunk so dependent
            # projection tiles can start before the whole tensor lands
            kxT_sb[b] = acts.tile([128, 2, S], BF, name=f"kx{b}", tag=f"kx{b}")
            vxT_sb[b] = acts.tile([128, 2, S], BF, name=f"vx{b}", tag=f"vx{b}")
            for cc in range(2):
                nc.sync.dma_start(out=kxT_sb[b][:, cc, :], in_=kxT_d[b, cc * 128:(cc + 1) * 128, :])
                nc.sync.dma_start(out=vxT_sb[b][:, cc, :], in_=vxT_d[b, cc * 128:(cc + 1) * 128, :])
            kT_sb[b] = [acts.tile([128, S], BF, name=f"kT{b}_{oc}", tag=f"kT{b}_{oc}") for oc in range(2)]
            qT_sb[b] = acts.tile([128, 2, QS], BF, name=f"qT{b}", tag=f"qT{b}")
            gT_sb[b] = acts.tile([128, 2, QS], BF, name=f"gT{b}", tag=f"gT{b}")
            v_sb[b] = [acts.tile([128, H, DH + 1], BF, name=f"v{b}_{kst}", tag=f"v{b}_{kst}") for kst in range(KST)]

        def emit_gate(b):
            # gate projection: only needed by the epilogue, so it is emitted
            # after the batch's attention loop to keep the startup prefix
            # short. gT2 is the head-major copy matching the osb layout.
            for oc in range(2):
                ps = aux_tile()
                for cc in range(2):
                    mm(ps[:, :QS], lhsT=w_sb["wg"][:, cc, oc * 128:(oc + 1) * 128],
                       rhs=qxT_sb[b][:, cc, :], start=(cc == 0), stop=(cc == 1))
                nc.scalar.activation(gT_sb[b][:, oc, :], ps[:, :QS], Sig,
                                     bias=bgt_sb[:, oc:oc + 1])
            gT2_sb[b] = acts.tile([DH, H, QS], BF, name=f"gT2{b}", tag=f"gT2{b}")
            for h in range(H):
                nc.gpsimd.dma_start(out=gT2_sb[b][:, h, :],
                                    in_=gT_sb[b][32 * (h % 4):32 * (h % 4) + 32, h // 4, :])

        # ---- phase 0: projections ----
        def emit_qproj(b):
            for oc in range(2):
                ps = aux_tile()
                for cc in range(2):
                    mm(ps[:, :QS], lhsT=w_sb["wq"][:, cc, oc * 128:(oc + 1) * 128],
                       rhs=qxT_sb[b][:, cc, :], start=(cc == 0), stop=(cc == 1))
                nc.vector.tensor_copy(qT_sb[b][:, oc, :], ps[:, :QS])

        def emit_kproj(b):
            for oc in range(2):
                for ns in range(4):
                    ps = aux_tile()
                    for cc in range(2):
                        mm(ps[:, :512], lhsT=w_sb["wk"][:, cc, oc * 128:(oc + 1) * 128],
                           rhs=kxT_sb[b][:, cc, ns * 512:(ns + 1) * 512],
                           start=(cc == 0), stop=(cc == 1))
                    # ACT is idle during the projection phase; route the k
                    # copies there so DVE keeps only the v/q copies
                    nc.scalar.copy(kT_sb[b][oc][:, ns * 512:(ns + 1) * 512], ps[:, :512])

        def emit_vproj(b):
            for kst2 in range(KST // 2):  # v (+ ones column per head)
                ps = aux_tile()
                for half in range(2):
                    kst = 2 * kst2 + half
                    for cc in range(2):
                        mm(ps[:, half * C:(half + 1) * C],
                           lhsT=vxT_sb[b][:, cc, kst * 128:(kst + 1) * 128],
                           rhs=w_sb["wv"][:, cc, :],
                           start=(half == 0 and cc == 0), stop=(half == 1 and cc == 1),
                           skip_group_check=True)
                for half in range(2):
                    kst = 2 * kst2 + half
                    va = v_sb[b][kst]
                    nc.vector.tensor_copy(va[:, :, 0:DH],
                                          ps[:, half * C:(half + 1) * C].rearrange("p (h c) -> p h c", c=DH))
                    nc.vector.memset(va[:, :, DH:DH + 1], 1.0)

        # batch 0 needs q, k, v before its attention; batch 1's q/k are
        # cheap and keep its attention start independent, but its v (the
        # most expensive projection) is deferred into batch 0's epilogue
        # window, shortening the startup prefix.
        emit_qproj(0)
        emit_kproj(0)
        emit_vproj(0)
        emit_qproj(1)
        emit_kproj(1)

        # ---- attention ----
        ofin = {}
        for b in range(B):
            if b == 1:
                emit_vproj(1)
            # PV accumulators. Column-packed: 2 one-bank tiles, head
            # h = g*4+s at rows (s%2)*64..+33 and free (s//2)*256 of tile g
            # (4 heads per bank; concurrent col-group writes hit different
            # partition groups, which the HW supports). Fallback: 4 tiles,
            # head 2p+hs at rows 0:33, free hs*256 of tile p.
            n_pv = 2 if PV_COL_PACK else 4
            pvt = [psPV.tile([128, 512], F32, name=f"pv{p}", tag="pv") for p in range(n_pv)]

            def pv_slot(h):
                if PV_COL_PACK:
                    g, s = h // 4, h % 4
                    return pvt[g], (s % 2) * 64, (s // 2) * 256, (s == 0), (s == 3)
                pair, hs = h // 2, h % 2
                return pvt[pair], 0, hs * 256, (hs == 0), (hs == 1)
            # Probs layout: head h = quad*4 + j lives at free offset
            # j*512 + quad*256 of the [128, 2048] qk/probs tiles, so the 4
            # concurrently-active row-tiled QK matmuls (row groups 32j) each
            # write a DIFFERENT psum bank (concurrent same-bank PE writes
            # hang the chip); the two quads reuse the same row groups and
            # therefore serialize on the PE.
            def emit_pv(probs, kst):
                # PV accumulates across the whole kst loop. At kst==0 the
                # slot-0 matmul's start=True zeroes its whole bank, which
                # races any concurrently-running col-group-64 matmul on the
                # same bank; issue order [s0, s2, s1, s3] keeps the odd col
                # groups strictly after the zeroing matmul has drained.
                heads = range(H)
                if PV_COL_PACK and kst == 0:
                    heads = [g * 4 + s for g in range(2) for s in (0, 2, 1, 3)]
                for h in heads:
                    tile_, row0, foff, first, last = pv_slot(h)
                    off = (h % 4) * 512 + (h // 4) * QS
                    mm(tile_[row0:row0 + 33, foff:foff + QS],
                       lhsT=v_sb[b][kst][:, h, :],
                       rhs=probs[:, off:off + QS],
                       start=(kst == 0 and first),
                       stop=(kst == KST - 1 and last),
                       tile_position=(0, row0),
                       skip_group_check=True)

            prev = None
            for kst in range(KST):
                bt = biasP.tile([128, 2048], BF, name="bias", tag="bias")
                nc.sync.dma_start(out=bt[:], in_=biasT_d[b, kst].rearrange("p a b q -> p (a b q)"))
                qk = psB.tile([128, 2048], F32, name="qk", tag="qk")
                for quad in range(2):
                    for j in range(4):
                        off = j * 512 + quad * QS
                        mm(qk[:, off:off + QS],
                           lhsT=kT_sb[b][quad][32 * j:32 * j + 32, kst * 128:(kst + 1) * 128],
                           rhs=qT_sb[b][32 * j:32 * j + 32, quad, :],
                           start=(quad == 0), stop=(quad == 1), tile_position=(32 * j, 0),
                           skip_group_check=True)
                probs = probsP.tile([128, 2048], BF, name="probs", tag="probs")
                nc.scalar.activation(probs[:], qk[:], Exp, bias=maskT_sb[:, b, kst:kst + 1])
                nc.vector.tensor_mul(probs[:], probs[:], bt[:])
                # PV for the previous kst is emitted after this kst's QK/exp
                # so the PE can start QK(kst+1) the moment ACT(kst) frees the
                # qk tile instead of sitting behind the probs multiply.
                if prev is not None:
                    emit_pv(*prev)
                prev = (probs, kst)
            emit_pv(*prev)
            emit_gate(b)

            # ---- epilogue part 1 for batch b (no PSUM, no sync queue) ----
            # Frees the pv psum slots immediately so the next batch's
            # attention can start; DMAs go on the gpsimd queue so the sync
            # queue keeps feeding the next batch's bias tiles.
            # osb[0:33, h*256:(h+1)*256] = head h's [o^T ; l] block
            osb = epiP.tile([33, 2048], F32, name=f"osb{b}", tag=f"osb{b}")
            if PV_COL_PACK:
                for h in range(H):
                    tile_, row0, foff, _, _ = pv_slot(h)
                    nc.vector.tensor_copy(osb[0:33, h * QS:(h + 1) * QS],
                                          tile_[row0:row0 + 33, foff:foff + QS])
            else:
                for pair in range(4):
                    nc.vector.tensor_copy(osb[0:33, pair * 512:(pair + 1) * 512], pvt[pair][0:33, :])
            # l lives in row 32 as one contiguous [1, 8*256] stripe: bounce it
            # through DRAM to fold it to [8, 256], reciprocal, bounce back,
            # then broadcast the whole 2048-wide stripe to all 32 d-rows.
            lb = dramP.tile([1, H * QS], F32, name=f"lb{b}", tag="lb")
            nc.sync.dma_start(out=lb[:], in_=osb[32:33, :])
            lsb = epiP.tile([H, QS], F32, name="lsb", tag="lsb")
            nc.sync.dma_start(out=lsb[:], in_=lb[0].rearrange("(h q) -> h q", q=QS))
            nc.vector.reciprocal(lsb[:], lsb[:])
            lb2 = dramP.tile([H, QS], F32, name=f"lb2{b}", tag="lb2")
            nc.sync.dma_start(out=lb2[:], in_=lsb[:])
            rep = epiP.tile([DH, H * QS], F32, name="rep", tag="rep")
            nc.sync.dma_start(out=rep[:],
                              in_=lb2.rearrange("h q -> (h q)")[None, :].broadcast_to([DH, H * QS]))
            nc.vector.tensor_mul(osb[0:DH, :], osb[0:DH, :], rep[:])
            of = epiP.tile([DH, H * QS], BF, name=f"ofin{b}", tag=f"ofin{b}")
            nc.vector.tensor_mul(of[:], osb[0:DH, :], gT2_sb[b].rearrange("d h q -> d (h q)"))
            ofin[b] = of

        # ---- output projections for both batches (pv slots are free) ----
        # out[qs, c] = sum_h ofin[0:DH, h*QS+qs].T @ woh[:, h, :]
        for b in range(B):
            for qc in range(2):
                ps = psB.tile([128, 2048], F32, name="ops", tag="qk")
                for h in range(H):
                    mm(ps[:, :C], lhsT=ofin[b][:, h * QS + qc * 128: h * QS + (qc + 1) * 128],
                       rhs=woh_sb[:, h, :], start=(h == 0), stop=(h == H - 1))
                outsb = epiP.tile([128, C], F32, name="outsb", tag="outsb")
                nc.vector.tensor_add(outsb[:], ps[:, :C], bo_sb[:])
                nc.sync.dma_start(out=out_d[b, qc * 128:(qc + 1) * 128, :], in_=outsb[:])

    nc.finalize()
    return nc


def _prep_inputs(q_x, k_x, v_x, bias_mask, bias_pair, Wq, Wk, Wv, Wg, bg, Wo, bo):
    scale = np.float32(1.0 / np.sqrt(DH))
    wqT = (Wq.astype(np.float32) * scale).T.copy().astype(BF16)
    wkT = Wk.T.copy().astype(BF16)
    wvT = Wv.T.copy().astype(BF16)
    wgT = Wg.T.copy().astype(BF16)
    woT = Wo.T.copy().astype(BF16)
    bgt = bg.astype(np.float32).reshape(2, 128).T.copy()
    bo2 = bo.astype(np.float32).reshape(1, C).copy()
    maskT = bias_mask.astype(np.float32).reshape(B, KST, 128).transpose(0, 2, 1).copy()
    kxT = k_x.transpose(0, 2, 1).copy().astype(BF16)
    vxT = v_x.transpose(0, 2, 1).copy().astype(BF16)

    # per-core tensors
    in_maps = []
    # biasT[core][b, kst, p, j, quad, qs] = exp(bias_pair)[b, h=quad*4+j,
    #                                                      core*QS+qs, kst*128+p]
    bp = bias_pair.transpose(0, 3, 1, 2)  # [b, k, h, q] view
    for i in range(N_CORES):
        qsl = slice(i * QS, (i + 1) * QS)
        qxT = q_x[:, qsl, :].transpose(0, 2, 1).copy().astype(BF16)
        biasT = np.exp(np.ascontiguousarray(bp[:, :, :, qsl]), dtype=np.float32)
        biasT = biasT.reshape(B, KST, 128, 2, 4, QS).swapaxes(4, 3).astype(BF16)
        biasT = np.ascontiguousarray(biasT)
        in_maps.append({
            "qxT": qxT, "kxT": kxT, "vxT": vxT,
            "wqT": wqT, "wkT": wkT, "wvT": wvT, "wgT": wgT, "woT": woT,
            "bgt": bgt, "bo": bo2, "maskT": maskT, "biasT": biasT,
        })
    return in_maps


def kernel(q_x, k_x, v_x, bias_mask, bias_pair, Wq, Wk, Wv, Wg, bg, Wo, bo):
    global LAST_RESULT
    from concourse.bass_utils import run_bass_kernel_spmd

    args = [np.asarray(a) for a in
            (q_x, k_x, v_x, bias_mask, bias_pair, Wq, Wk, Wv, Wg, bg, Wo, bo)]
    if "nc" not in _CACHE:
        _CACHE["nc"] = _build_graph()
    nc = _CACHE["nc"]
    in_maps = _prep_inputs(*args)
    res = run_bass_kernel_spmd(
        nc, in_maps, core_ids=list(range(N_CORES)),
        trace=bool(os.environ.get("KERNEL_TRACE")),
    )
    LAST_RESULT = res
    out = np.concatenate([res.results[i]["out"] for i in range(N_CORES)], axis=1)
    return out.astype(np.float32)
